# revision 6
# baseline (speedup 1.0000x reference)
"""BNAF forward (B=2048, D=8, H=512, 4 masked layers) on 8 TRN2 NeuronCores.

Strategy
--------
Pure data parallel: batch is split 256/core; the small weights are replicated.

Math: the BNAF log-det recursion collapses in exp space.  For each masked
linear layer, exp(logdet diag blocks) == the diag blocks of the normalized
weight w itself, and for tanh, exp(logdet) == 1 - h^2.  So the whole
log-sum-exp flow is a chain of *positive* block-diagonal matmuls with one
log() at the very end.  The per-output norm scale s = exp(logg)/||v|| is
folded into the NEXT layer's G-flow weights (input-side, baseline wd form).

v2 redesign vs the original baseline:
- no PE warmup matmuls (8.8us of PE queue); no second ACT table: the final
  Ln is a 2-op DVE fast-log (bitcast + affine), abs err ~0.03 << tol.
- fp16 weights shipped from host (same bits the old DMA-cast produced),
  2 big strided DMAs per layer instead of 4 windowed ones, split queues.
- dummy ACT at t0 pulls the one exp_and_others table load into the DMA
  window.
- norm path: v^2 pieces on GpSimd (idle), row-sums via ones-matmuls with
  early off-diag pieces (raw W, no exp dependency), K=1 matmul columnize,
  one-step Newton rsqrt.
- G-flow matmuls read a separate [128,512] block-diag wdG tile (s-folded
  per-partition at copy time); G psum is contiguous so the sech2 fold is
  one full-width DVE op per layer.
- elementwise work split across Vector/GpSimd/Scalar to balance engines.
"""

import numpy as np

TRACE = False          # set by test.py for profiling runs
LAST_RESULTS = None    # BassKernelResults stash for test.py

_CACHE = {}

P = 128
BC = 256          # batch per core
H = 512
NCORE = 8
MAGIC = 0x5f3759df

# smalls layout: first the exp-batch block (exp'd in one ACT op), then rest
_SM = {}
_off = 0
for _name, _w in [("w1dg", 4), ("w4dg", 4), ("lg1", 4), ("lg2", 4),
                  ("lg3", 4), ("lg4c", 1),                      # <- exp block
                  ("b4c", 1), ("b1", 4), ("b2", 4), ("b3", 4),
                  ("ident", 128), ("w1n", 32), ("w4t", 32),
                  ("md1n", 32), ("mo1n", 32), ("md4t", 32), ("mo4t", 32)]:
    _SM[_name] = (_off, _off + _w)
    _off += _w
SMALL_W = _off
EXPW = _SM["lg4c"][1]          # width of the exp block (21)

FASTLN_A = float(np.log(2.0) / (1 << 23))
FASTLN_B = float((0.0430 - 127.0) * np.log(2.0))


def _build():
    import concourse.bacc as bacc
    import concourse.mybir as mybir
    import concourse.tile as tile
    from concourse.bass import AP
    from contextlib import ExitStack

    f32 = mybir.dt.float32
    u32 = mybir.dt.uint32
    i32 = mybir.dt.int32
    bf16 = mybir.dt.bfloat16
    fp16 = mybir.dt.float16
    E = mybir.ActivationFunctionType
    ALU = mybir.AluOpType

    nc = bacc.Bacc("TRN2", target_bir_lowering=False, debug=False,
                   enable_asserts=False, num_devices=NCORE)

    t = {}
    t["xT"] = nc.dram_tensor("xT", (8, BC), fp16, kind="ExternalInput").ap()
    t["w2T"] = nc.dram_tensor("w2T", (H, H), fp16, kind="ExternalInput").ap()
    t["w3T"] = nc.dram_tensor("w3T", (H, H), fp16, kind="ExternalInput").ap()
    t["smalls"] = nc.dram_tensor("smalls", (P, SMALL_W), f32, kind="ExternalInput").ap()
    t["hT_out"] = nc.dram_tensor("hT_out", (8, BC), f32, kind="ExternalOutput").ap()
    t["sldT_out"] = nc.dram_tensor("sldT_out", (8, BC), f32, kind="ExternalOutput").ap()

    def mm(out, lhsT, rhs, **kw):
        nc.tensor.matmul(out, lhsT, rhs, **kw)

    def winap(base_tile, p0, np_, col0, n, stride, w):
        """[np_ parts at p0] x (n windows of w cols, stride apart, from col0)."""
        base = base_tile[p0:p0 + np_, col0:col0 + w]
        return AP(base.tensor, base.offset,
                  [[base.ap[0][0], np_], [stride, n], [1, w]])

    with tile.TileContext(nc) as tc, ExitStack() as ctx:
        wgt = ctx.enter_context(tc.tile_pool(name="wgt", bufs=1))
        scr = ctx.enter_context(tc.tile_pool(name="scr", bufs=4))
        psA = ctx.enter_context(tc.tile_pool(name="psA", bufs=3, space="PSUM"))
        psN = ctx.enter_context(tc.tile_pool(name="psN", bufs=2, space="PSUM"))

        act = nc.scalar.activation
        cp = nc.vector.tensor_copy
        ts = nc.vector.tensor_scalar
        stt = nc.vector.scalar_tensor_tensor
        mul = nc.vector.tensor_mul
        tt = nc.vector.tensor_tensor
        gtt = nc.gpsimd.tensor_tensor
        gms = nc.gpsimd.memset

        # ---- dummy ACT at t0: pulls the exp_and_others table load ---------
        dmy = wgt.tile([P, 1], f32, name="dmy", tag="dmy")
        dmyo = wgt.tile([P, 1], f32, name="dmyo", tag="dmyo")
        with tc.high_priority():
            nc.vector.memset(dmy, 0.0)
            act(dmyo, dmy, E.Exp)

        # ---- input DMAs --------------------------------------------------
        smalls = wgt.tile([P, SMALL_W], f32, name="smalls_t", tag="smalls_t")
        xT = wgt.tile([8, BC], fp16, name="xT_t", tag="xT_t")
        vt_t = {}
        for l in (2, 3):
            vt_t[l] = wgt.tile([P, 4 * H], fp16, name=f"vt{l}", tag=f"vt{l}")
        with tc.high_priority():
            nc.sync.dma_start(smalls, t["smalls"])
            nc.sync.dma_start(xT, t["xT"])
            # per layer: 2 strided DMAs (chunk pairs), full-width rows.
            # src rows 128k..128k+128 of [512,512] land at vt cols 512k..+512.
            for l, q in ((2, nc.sync), (3, nc.gpsimd)):
                for hh in (0, 1):
                    src = AP(t[f"w{l}T"].tensor, hh * 2 * P * H,
                             [[H, P], [P * H, 2], [1, H]])
                    q.dma_start(vt_t[l][:, 2 * H * hh:2 * H * hh + 2 * H], src)

        def sm(name):
            a, b = _SM[name]
            return smalls[:, a:b]

        ident = sm("ident")
        lg4 = smalls[0:8, _SM["lg4c"][0]:_SM["lg4c"][1]]
        b4 = smalls[0:8, _SM["b4c"][0]:_SM["b4c"][1]]

        # ---- constants ---------------------------------------------------
        ones2f = wgt.tile([P, 2], f32, name="ones2f", tag="ones2f")
        ones2 = wgt.tile([P, 2], fp16, name="ones2", tag="ones2")
        magict = wgt.tile([P, 4], u32, name="magict", tag="magict")
        with tc.high_priority():
            gms(ones2f, 1.0)
            gms(magict, MAGIC)
            cp(ones2, ones2f)

        # ---- batched exps over smalls ------------------------------------
        esm = wgt.tile([P, EXPW], f32, name="esm", tag="esm")
        e14 = wgt.tile([P, 64], f32, name="e14", tag="e14")
        with tc.high_priority():
            act(esm, smalls[:, 0:EXPW], E.Exp)
            act(e14, smalls[:, _SM["w1n"][0]:_SM["w4t"][1]], E.Exp)
        e1n = e14[:, 0:32]
        e4t = e14[:, 32:64]
        e1d = esm[:, _SM["w1dg"][0]:_SM["w1dg"][1]]
        eg = {1: esm[:, _SM["lg1"][0]:_SM["lg1"][1]],
              2: esm[:, _SM["lg2"][0]:_SM["lg2"][1]],
              3: esm[:, _SM["lg3"][0]:_SM["lg3"][1]]}
        eg4 = esm[0:8, _SM["lg4c"][0]:_SM["lg4c"][1]]

        # s = eg * rsqrt(n2): magic seed + one Newton step
        def make_scale(n2_ap, eg_ap, shape, nm):
            pr = shape[0]
            n2s = scr.tile(list(shape), f32, name=f"n2s_{nm}", tag="sc_n2s")
            cp(n2s, n2_ap)
            shf = scr.tile(list(shape), u32, name=f"shf_{nm}", tag="sc_shf")
            ts(shf, n2s.bitcast(u32), 1, None, op0=ALU.arith_shift_right)
            y0 = scr.tile(list(shape), u32, name=f"y0_{nm}", tag="sc_y0")
            stt(y0, magict[:pr, :shape[1]], 0, shf, op0=ALU.bypass, op1=ALU.subtract)
            y = y0.bitcast(f32)
            t1 = scr.tile(list(shape), f32, name=f"t1_{nm}", tag="sc_t1")
            t2 = scr.tile(list(shape), f32, name=f"t2_{nm}", tag="sc_t2")
            mul(t1, y, y)
            mul(t2, t1, n2s)
            ts(t1, t2, -0.5, 1.5, op0=ALU.mult, op1=ALU.add)
            yn = scr.tile(list(shape), f32, name=f"yn_{nm}", tag="sc_yn")
            mul(yn, y, t1)
            s = wgt.tile(list(shape), f32, name=f"s_{nm}", tag=f"s_{nm}")
            mul(s, eg_ap, yn)
            return s

        # ================= layer 2/3 prep part A (norms) ==================
        # diag exp in place, LL zero, v^2 pieces, row-sums, columnize, scale
        def big_prep(l):
            vt = vt_t[l]
            dA = winap(vt, 0, 64, 0, 4, 640, 64)
            dB = winap(vt, 64, 64, 64, 4, 640, 64)
            dLL = winap(vt, 64, 64, 0, 4, 640, 64)
            with tc.high_priority():
                gms(dLL, 0.0)
                act(dA, dA, E.Exp)
                act(dB, dB, E.Exp)
            vsq = scr.tile([P, 4 * H], fp16, name=f"vsq{l}", tag="vsq")
            with tc.high_priority():
                # off-diag pieces: raw W, only need the DMA
                for k in range(3):
                    a, b = 640 * k + P, H * (k + 1)
                    gtt(vsq[:, a:b], vt[:, a:b], vt[:, a:b], op=ALU.mult)
                # diag piece: needs exp + LL memset
                vqd = winap(vsq, 0, P, 0, 4, 640, P)
                vtd = winap(vt, 0, P, 0, 4, 640, P)
                gtt(vqd, vtd, vtd, op=ALU.mult)
            nrow = psN.tile([2, H], f32, name=f"nrow{l}", tag="pn")
            with tc.high_priority():
                for k in range(3):
                    a, b = 640 * k + P, H * (k + 1)
                    mm(nrow[:, P * (k + 1):H], ones2, vsq[:, a:b],
                       start=(k == 0), stop=False, skip_group_check=True)
                # start=False everywhere: has_written is 0 on untouched
                # elements (off0's bank-clear), so these write-or-accumulate
                # per element as needed.
                for k in range(4):
                    mm(nrow[:, P * k:P * k + P], ones2, vsq[:, 640 * k:640 * k + P],
                       start=False, stop=(k == 3), skip_group_check=True)
                nrs = scr.tile([1, H], f32, name=f"nrs{l}", tag="nrs")
                act(nrs, nrow[0:1, :], E.Copy)
                ncol = psN.tile([P, 4], f32, name=f"ncol{l}", tag="pn")
                for c in range(4):
                    mm(ncol[:, c:c + 1], nrs[0:1, P * c:P * c + P], ones2f[0:1, 0:1])
                s = make_scale(ncol, eg[l], (P, 4), f"l{l}")
            return s

        # wdG: [dA 0; 0 dB] with input-side s_prev fold (per-partition)
        def make_wdG(l, s_prev):
            vt = vt_t[l]
            wdG = wgt.tile([P, 4 * P], fp16, name=f"wdG{l}", tag=f"wdG{l}")
            for c in range(4):
                ts(wdG[:, P * c:P * c + P], vt[:, 640 * c:640 * c + P],
                   s_prev[:, c:c + 1], None, op0=ALU.mult)
            gms(winap(wdG, 0, 64, 64, 4, P, 64), 0.0)   # zero the X quadrant
            return wdG

        s2 = big_prep(2)

        # ================= layer 1 prep (natural layout [512,8]) ==========
        v1n = wgt.tile([P, 32], f32, name="v1n", tag="v1n")
        n1 = scr.tile([P, 4], f32, name="n1", tag="n1")
        vT1 = wgt.tile([8, H], fp16, name="vT1", tag="vT1")
        with tc.high_priority():
            v1a = scr.tile([P, 32], f32, name="v1a", tag="v1a")
            mul(v1a, e1n, sm("md1n"))
            v1b = scr.tile([P, 32], f32, name="v1b", tag="v1b")
            mul(v1b, sm("w1n"), sm("mo1n"))
            tt(v1n, v1a, v1b, op=ALU.add)
            for c in range(4):
                sq1 = scr.tile([P, 8], f32, name=f"sq1_{c}", tag="sq1")
                stt(sq1, v1n[:, 8 * c:8 * c + 8], 0, v1n[:, 8 * c:8 * c + 8],
                    op0=ALU.bypass, op1=ALU.mult, accum_out=n1[:, c:c + 1])
            s1 = make_scale(n1, eg[1], (P, 4), "l1")
            for c in range(4):
                pt = psN.tile([8, P], f32, name=f"pt1_{c}", tag="pn")
                nc.tensor.transpose(pt, v1n[:, 8 * c:8 * c + 8], ident)
                cp(vT1[:, P * c:P * c + P], pt)

        # ================= layer 1 batch ==================================
        h1 = wgt.tile([P, 4 * BC], fp16, name="h1", tag="h1")
        z1 = psA.tile([P, 4 * BC], f32, name="z1", tag="ps")
        with tc.high_priority():
            for c in range(4):
                mm(z1[:, BC * c:BC * c + BC], vT1[:, P * c:P * c + P], xT)
                act(h1[:, BC * c:BC * c + BC], z1[:, BC * c:BC * c + BC], E.Tanh,
                    bias=sm("b1")[:, c:c + 1], scale=s1[:, c:c + 1])
        hq1 = scr.tile([P, 4 * BC], fp16, name="hq1", tag="hq")
        gtt(hq1, h1, h1, op=ALU.mult)
        sc1 = scr.tile([P, 4 * BC], fp16, name="sc1", tag="sech2")
        ts(sc1, hq1, -1.0, 1.0, op0=ALU.mult, op1=ALU.add)
        F1 = wgt.tile([P, 4 * BC], bf16, name="F1", tag="F1")
        for c in range(4):
            ts(F1[:, BC * c:BC * c + BC], sc1[:, BC * c:BC * c + BC],
               e1d[:, c:c + 1], None, op0=ALU.mult)

        wdG2 = make_wdG(2, s1)
        s3 = big_prep(3)

        # ================= layer 2/3 batch ================================
        def big_batch(l, s, wdG, h_prev, F_prev):
            vt = vt_t[l]
            hl = wgt.tile([P, 4 * BC], fp16, name=f"h{l}", tag=f"h{l}")
            zh = psA.tile([P, 4 * BC], f32, name=f"zh{l}", tag="ps")
            zg = psA.tile([P, 4 * BC], f32, name=f"zg{l}", tag="ps")
            for c in range(4):
                for k in range(c + 1):
                    mm(zh[:, BC * c:BC * c + BC],
                       vt[:, H * k + P * c:H * k + P * c + P],
                       h_prev[:, BC * k:BC * k + BC],
                       start=(k == 0), stop=(k == c))
                act(hl[:, BC * c:BC * c + BC], zh[:, BC * c:BC * c + BC], E.Tanh,
                    bias=sm(f"b{l}")[:, c:c + 1], scale=s[:, c:c + 1])
                mm(zg[:, BC * c:BC * c + BC], wdG[:, P * c:P * c + P],
                   F_prev[:, BC * c:BC * c + BC], start=True, stop=True)
            hql = scr.tile([P, 4 * BC], fp16, name=f"hq{l}", tag="hq")
            gtt(hql, hl, hl, op=ALU.mult)
            scl = scr.tile([P, 4 * BC], fp16, name=f"sc{l}", tag="sech2")
            ts(scl, hql, -1.0, 1.0, op0=ALU.mult, op1=ALU.add)
            Fl = wgt.tile([P, 4 * BC], bf16, name=f"F{l}", tag=f"F{l}")
            tt(Fl, scl, zg, op=ALU.mult)
            return hl, Fl

        h2, F2 = big_batch(2, s2, wdG2, h1, F1)
        wdG3 = make_wdG(3, s2)

        # ================= layer 4 prep (transposed [512,8]) ==============
        vt4 = wgt.tile([P, 32], fp16, name="vt4", tag="vt4")
        v4a = scr.tile([P, 32], f32, name="v4a", tag="v4a")
        mul(v4a, e4t, sm("md4t"))
        v4b = scr.tile([P, 32], f32, name="v4b", tag="v4b")
        mul(v4b, sm("w4t"), sm("mo4t"))
        tt(vt4, v4a, v4b, op=ALU.add)
        vsq4 = scr.tile([P, 32], fp16, name="vsq4", tag="vsq4")
        act(vsq4, vt4, E.Square)
        pn4 = psN.tile([8, 2], f32, name="pn4", tag="pn")
        for k in range(4):
            mm(pn4, vsq4[:, 8 * k:8 * k + 8], ones2, start=(k == 0), stop=(k == 3))
        s4 = make_scale(pn4[:, 0:1], eg4, (8, 1), "l4")

        h3, F3 = big_batch(3, s3, wdG3, h2, F2)

        vd4 = wgt.tile([P, 32], bf16, name="vd4", tag="vd4")
        for k in range(4):
            ts(vd4[:, 8 * k:8 * k + 8], v4a[:, 8 * k:8 * k + 8],
               s3[:, k:k + 1], None, op0=ALU.mult)

        # ================= layer 4 batch ==================================
        z4 = psA.tile([P, 4 * BC], f32, name="z4", tag="ps")
        pz4 = z4[0:8, 0:BC]
        pf4 = z4[0:8, BC:2 * BC]
        for k in range(4):
            mm(pz4, vt4[:, 8 * k:8 * k + 8], h3[:, BC * k:BC * k + BC],
               start=(k == 0), stop=(k == 3))
        h4 = wgt.tile([8, BC], f32, name="h4", tag="h4")
        act(h4, pz4, E.Tanh, bias=b4, scale=s4)
        nc.sync.dma_start(t["hT_out"], h4)
        for k in range(4):
            mm(pf4, vd4[:, 8 * k:8 * k + 8], F3[:, BC * k:BC * k + BC],
               start=(k == 0), stop=(k == 3))
        hq4 = scr.tile([8, BC], f32, name="hq4", tag="hq4")
        mul(hq4, h4, h4)
        s24 = scr.tile([8, BC], f32, name="s24", tag="s24")
        ts(s24, hq4, -1.0, 1.0, op0=ALU.mult, op1=ALU.add)
        gt = scr.tile([8, BC], f32, name="gt", tag="gt")
        stt(gt, pf4, s4, s24, op0=ALU.mult, op1=ALU.mult)
        # fast natural log: ln(x) ~= A*bits(x) + B   (max abs err ~0.03)
        gbits = scr.tile([8, BC], f32, name="gbits", tag="gbits")
        cp(gbits, gt.bitcast(i32))
        sld = wgt.tile([8, BC], f32, name="sld", tag="sld")
        ts(sld, gbits, FASTLN_A, FASTLN_B, op0=ALU.mult, op1=ALU.add)
        nc.gpsimd.dma_start(t["sldT_out"], sld)

    nc.compile()
    return nc


def _host_prep(x, W1, logg1, bias1, W2, logg2, bias2, W3, logg3, bias3,
               W4, logg4, bias4):
    """Pure layout prep (transpose / reshape / gather / masks / casts)."""
    f = np.float32

    def cols(a):          # [512]-ish vector -> [128, 4] column-chunk layout
        return np.ascontiguousarray(np.reshape(a, (4, P)).T).astype(f)

    smalls = np.zeros((P, SMALL_W), f)

    def put(name, arr):
        a, b = _SM[name]
        smalls[:arr.shape[0], a:b] = arr

    def fold(m):          # [512, 8] -> [128, (k x)] with k = row-chunk
        return m.reshape(4, P, 8).transpose(1, 0, 2).reshape(P, 32)

    put("ident", np.eye(P, dtype=f))
    put("w1n", fold(W1))                                   # natural [512,8]
    put("w4t", fold(np.ascontiguousarray(W4.T)))           # [512,8]
    put("w1dg", cols(W1[np.arange(H), np.arange(H) // 64]))
    put("w4dg", cols(W4[np.arange(H) // 64, np.arange(H)]))
    put("lg1", cols(logg1)); put("b1", cols(bias1))
    put("lg2", cols(logg2)); put("b2", cols(bias2))
    put("lg3", cols(logg3)); put("b3", cols(bias3))
    put("lg4c", logg4.reshape(8, 1).astype(f))
    put("b4c", bias4.reshape(8, 1).astype(f))
    # structural masks
    o = np.arange(H)[:, None] // 64
    i1 = np.arange(8)[None, :]
    md1 = (i1 == o).astype(f); mo1 = (i1 < o).astype(f)    # [512, 8] natural
    put("md1n", fold(md1)); put("mo1n", fold(mo1))
    ii = np.arange(H)[:, None] // 64
    o4 = np.arange(8)[None, :]
    md4 = (o4 == ii).astype(f); mo4 = (o4 > ii).astype(f)  # [512, 8] transposed
    put("md4t", fold(md4)); put("mo4t", fold(mo4))

    w2T = np.ascontiguousarray(W2.T).astype(np.float16)
    w3T = np.ascontiguousarray(W3.T).astype(np.float16)
    xT = np.ascontiguousarray(x.T).astype(np.float16)      # [8, 2048]
    return xT, w2T, w3T, smalls


def kernel(**inputs):
    global LAST_RESULTS
    from concourse.bass_utils import run_bass_kernel_spmd

    xT, w2T, w3T, smalls = _host_prep(**{k: np.asarray(v) for k, v in inputs.items()})

    if "nc" not in _CACHE:
        _CACHE["nc"] = _build()
    nc = _CACHE["nc"]

    in_maps = []
    for c in range(NCORE):
        in_maps.append({
            "xT": np.ascontiguousarray(xT[:, BC * c:BC * (c + 1)]),
            "w2T": w2T, "w3T": w3T, "smalls": smalls,
        })
    res = run_bass_kernel_spmd(nc, in_maps, core_ids=list(range(NCORE)),
                               trace=TRACE)
    LAST_RESULTS = res

    B = BC * NCORE
    h = np.empty((B, 8), np.float32)
    sld = np.empty((B, 8), np.float32)
    for c, r in enumerate(res.results):
        h[BC * c:BC * (c + 1)] = r["hT_out"].T
        sld[BC * c:BC * (c + 1)] = r["sldT_out"].T
    return h, sld


# revision 8
# speedup vs baseline: 1.2438x; 1.2438x over previous
"""BNAF forward (B=2048, D=8, H=512, 4 masked layers) on 8 TRN2 NeuronCores.

Strategy
--------
Pure data parallel: batch is split 256/core; the small weights are replicated.

Math: the BNAF log-det recursion collapses in exp space.  For each masked
linear layer, exp(logdet diag blocks) == the diag blocks of the normalized
weight w itself, and for tanh, exp(logdet) == 1 - h^2.  So the whole
log-sum-exp flow is a chain of *positive* block-diagonal matmuls with one
log() at the very end.  The per-output norm scale s = exp(logg)/||v|| is
folded into the NEXT layer's G-flow weights (input-side, wd form).

v3 layout: one [128, 4608] combined weight tile holds both big layers
(w3 at col 2560) so all 8 diag 128-blocks sit at uniform 640-stride and
one windowed ACT exps them all.  No warmup matmuls, one ACT table, final
Ln is a 2-op DVE fast-log, fp16 weights shipped from host, fp16 norm
columnize matmuls, elementwise work split across Vector/GpSimd.
"""

import numpy as np

TRACE = False          # set by test.py for profiling runs
LAST_RESULTS = None    # BassKernelResults stash for test.py

_CACHE = {}

P = 128
BC = 256          # batch per core
H = 512
NCORE = 8
MAGIC = 0x5f3759df
L3OFF = 2560      # col offset of layer-3 weights in the combined vt tile

# smalls layout: first the exp-batch block (one ACT op), then the rest
_SM = {}
_off = 0
for _name, _w in [("w1dg", 4), ("w4dg", 4), ("lg1", 4), ("lg2", 4),
                  ("lg3", 4), ("lg4c", 1), ("w1n", 32), ("w4t", 32),  # exp blk
                  ("b4c", 1), ("b1", 4), ("b2", 4), ("b3", 4),
                  ("ident", 128),
                  ("md1n", 32), ("mo1n", 32), ("md4t", 32), ("mo4t", 32)]:
    _SM[_name] = (_off, _off + _w)
    _off += _w
SMALL_W = _off
EXPW = _SM["w4t"][1]           # width of the exp block (85)

FASTLN_A = float(np.log(2.0) / (1 << 23))
FASTLN_B = float((0.0430 - 127.0) * np.log(2.0))


def _build():
    import concourse.bacc as bacc
    import concourse.mybir as mybir
    import concourse.tile as tile
    from concourse.bass import AP
    from contextlib import ExitStack

    f32 = mybir.dt.float32
    u32 = mybir.dt.uint32
    i32 = mybir.dt.int32
    bf16 = mybir.dt.bfloat16
    fp16 = mybir.dt.float16
    E = mybir.ActivationFunctionType
    ALU = mybir.AluOpType

    nc = bacc.Bacc("TRN2", target_bir_lowering=False, debug=False,
                   enable_asserts=False, num_devices=NCORE)

    t = {}
    t["xT"] = nc.dram_tensor("xT", (8, BC), fp16, kind="ExternalInput").ap()
    t["w2T"] = nc.dram_tensor("w2T", (H, H), fp16, kind="ExternalInput").ap()
    t["w3T"] = nc.dram_tensor("w3T", (H, H), fp16, kind="ExternalInput").ap()
    t["smalls"] = nc.dram_tensor("smalls", (P, SMALL_W), f32, kind="ExternalInput").ap()
    t["hT_out"] = nc.dram_tensor("hT_out", (8, BC), f32, kind="ExternalOutput").ap()
    t["sldT_out"] = nc.dram_tensor("sldT_out", (8, BC), f32, kind="ExternalOutput").ap()

    def mm(out, lhsT, rhs, **kw):
        nc.tensor.matmul(out, lhsT, rhs, **kw)

    def winap(base_tile, p0, np_, col0, n, stride, w):
        """[np_ parts at p0] x (n windows of w cols, stride apart, from col0)."""
        base = base_tile[p0:p0 + np_, col0:col0 + w]
        return AP(base.tensor, base.offset,
                  [[base.ap[0][0], np_], [stride, n], [1, w]])

    with tile.TileContext(nc) as tc, ExitStack() as ctx:
        wgt = ctx.enter_context(tc.tile_pool(name="wgt", bufs=1))
        scr = ctx.enter_context(tc.tile_pool(name="scr", bufs=4))
        psA = ctx.enter_context(tc.tile_pool(name="psA", bufs=3, space="PSUM"))
        psN = ctx.enter_context(tc.tile_pool(name="psN", bufs=2, space="PSUM"))

        act = nc.scalar.activation
        cp = nc.vector.tensor_copy
        ts = nc.vector.tensor_scalar
        stt = nc.vector.scalar_tensor_tensor
        mul = nc.vector.tensor_mul
        tt = nc.vector.tensor_tensor
        gtt = nc.gpsimd.tensor_tensor
        gms = nc.gpsimd.memset

        # ---- dummy ACT at t0 pulls the single exp_and_others table load ---
        dmy = wgt.tile([P, 1], f32, name="dmy", tag="dmy")
        dmyo = wgt.tile([P, 1], f32, name="dmyo", tag="dmyo")
        nc.vector.memset(dmy, 0.0)
        act(dmyo, dmy, E.Exp)

        # ---- input DMAs ---------------------------------------------------
        smalls = wgt.tile([P, SMALL_W], f32, name="smalls_t", tag="smalls_t")
        xT = wgt.tile([8, BC], fp16, name="xT_t", tag="xT_t")
        # combined weight tile: w2 at col 0, w3 at col L3OFF; diag 128-block
        # j (j=0..7) sits at cols [640j, 640j+128)
        vt = wgt.tile([P, L3OFF + 4 * H], fp16, name="vt", tag="vt")
        nc.sync.dma_start(smalls, t["smalls"])
        for l, q, base in ((2, nc.sync, 0), (3, nc.gpsimd, L3OFF)):
            for hh in (0, 1):
                src = AP(t[f"w{l}T"].tensor, hh * 2 * P * H,
                         [[H, P], [P * H, 2], [1, H]])
                q.dma_start(vt[:, base + 2 * H * hh:base + 2 * H * hh + 2 * H], src)
        nc.sync.dma_start(xT, t["xT"])

        def vcol(l, k):        # start col of (layer, row-chunk k) in vt
            return (0 if l == 2 else L3OFF) + H * k

        def sm(name):
            a, b = _SM[name]
            return smalls[:, a:b]

        ident = sm("ident")
        b4 = smalls[0:8, _SM["b4c"][0]:_SM["b4c"][1]]

        # ---- constants ----------------------------------------------------
        ones2f = wgt.tile([P, 2], f32, name="ones2f", tag="ones2f")
        ones2 = wgt.tile([P, 2], fp16, name="ones2", tag="ones2")
        magict = wgt.tile([P, 4], u32, name="magict", tag="magict")
        gms(ones2f, 1.0)
        gms(magict, MAGIC)
        cp(ones2, ones2f)

        # ---- diag prep: zero LL, exp the 8 diag 64-blocks (2 ACTs) -------
        dLL = winap(vt, 64, 64, 0, 8, 640, 64)
        gms(dLL, 0.0)
        dA = winap(vt, 0, 64, 0, 8, 640, 64)
        dB = winap(vt, 64, 64, 64, 8, 640, 64)
        act(dA, dA, E.Exp)
        act(dB, dB, E.Exp)

        # ---- one exp over all the small weight pieces ---------------------
        esm = wgt.tile([P, EXPW], f32, name="esm", tag="esm")
        act(esm, smalls[:, 0:EXPW], E.Exp)
        e1n = esm[:, _SM["w1n"][0]:_SM["w1n"][1]]
        e4t = esm[:, _SM["w4t"][0]:_SM["w4t"][1]]
        e1d = esm[:, _SM["w1dg"][0]:_SM["w1dg"][1]]
        eg = {1: esm[:, _SM["lg1"][0]:_SM["lg1"][1]],
              2: esm[:, _SM["lg2"][0]:_SM["lg2"][1]],
              3: esm[:, _SM["lg3"][0]:_SM["lg3"][1]]}
        eg4 = esm[0:8, _SM["lg4c"][0]:_SM["lg4c"][1]]

        # ---- v^2 pieces ---------------------------------------------------
        # off-diag (raw W, DMA-gated only): 6 pieces on GpSimd; the L2 ones
        # first.  Diag windows (exp-gated): one full-width DVE op.
        vsq = scr.tile([P, L3OFF + 4 * H], fp16, name="vsq", tag="vsq")
        # piece k of a layer: cols base+640k+128 .. base+512(k+1)
        for k in range(3):
            a, b = 640 * k + P, H * (k + 1)
            gtt(vsq[:, a:b], vt[:, a:b], vt[:, a:b], op=ALU.mult)
        for k in range(3):
            a, b = L3OFF + 640 * k + P, L3OFF + H * (k + 1)
            gtt(vsq[:, a:b], vt[:, a:b], vt[:, a:b], op=ALU.mult)
        vqd = winap(vsq, 0, P, 0, 8, 640, P)
        vtd = winap(vt, 0, P, 0, 8, 640, P)
        tt(vqd, vtd, vtd, op=ALU.mult)      # on Vector: critical for s2

        # s = eg * rsqrt(n2): magic seed + one Newton step
        def make_scale(n2_ap, eg_ap, shape, nm):
            pr = shape[0]
            n2s = scr.tile(list(shape), f32, name=f"n2s_{nm}", tag="sc_n2s")
            cp(n2s, n2_ap)
            shf = scr.tile(list(shape), u32, name=f"shf_{nm}", tag="sc_shf")
            ts(shf, n2s.bitcast(u32), 1, None, op0=ALU.arith_shift_right)
            y0 = scr.tile(list(shape), u32, name=f"y0_{nm}", tag="sc_y0")
            stt(y0, magict[:pr, :shape[1]], 0, shf, op0=ALU.bypass, op1=ALU.subtract)
            y = y0.bitcast(f32)
            t1 = scr.tile(list(shape), f32, name=f"t1_{nm}", tag="sc_t1")
            t2 = scr.tile(list(shape), f32, name=f"t2_{nm}", tag="sc_t2")
            mul(t1, y, y)
            mul(t2, t1, n2s)
            ts(t1, t2, -0.5, 1.5, op0=ALU.mult, op1=ALU.add)
            yn = scr.tile(list(shape), f32, name=f"yn_{nm}", tag="sc_yn")
            mul(yn, y, t1)
            s = wgt.tile(list(shape), f32, name=f"s_{nm}", tag=f"s_{nm}")
            mul(s, eg_ap, yn)
            return s

        # ---- norm row-sums + columnize + scale, per big layer ------------
        def norms(l, nrow_first):
            base = 0 if l == 2 else L3OFF
            nrow = psN.tile([2, H], f32, name=f"nrow{l}", tag="pn")
            for k in range(3):
                a, b = base + 640 * k + P, base + H * (k + 1)
                mm(nrow[:, P * (k + 1):H], ones2, vsq[:, a:b],
                   start=(k == 0), stop=False, skip_group_check=True)
            for k in range(4):
                mm(nrow[:, P * k:P * k + P], ones2,
                   vsq[:, base + 640 * k:base + 640 * k + P],
                   start=False, stop=(k == 3), skip_group_check=True)
            nrs = scr.tile([1, H], fp16, name=f"nrs{l}", tag="nrs")
            act(nrs, nrow[0:1, :], E.Copy)
            ncol = psN.tile([P, 4], f32, name=f"ncol{l}", tag="pn")
            for c in range(4):
                mm(ncol[:, c:c + 1], nrs[0:1, P * c:P * c + P], ones2[0:1, 0:1])
            return make_scale(ncol, eg[l], (P, 4), f"l{l}")

        # wdG: [dA 0; 0 dB] with input-side s_prev fold (per-partition)
        def make_wdG(l, s_prev):
            base = 0 if l == 2 else L3OFF
            wdG = wgt.tile([P, 4 * P], fp16, name=f"wdG{l}", tag=f"wdG{l}")
            for c in range(4):
                ts(wdG[:, P * c:P * c + P], vt[:, base + 640 * c:base + 640 * c + P],
                   s_prev[:, c:c + 1], None, op0=ALU.mult)
            gms(winap(wdG, 0, 64, 64, 4, P, 64), 0.0)   # zero the X quadrant
            return wdG

        # ================= layer 1 prep (natural layout [512,8]) ==========
        v1n = wgt.tile([P, 32], f32, name="v1n", tag="v1n")
        n1 = scr.tile([P, 4], f32, name="n1", tag="n1")
        vT1 = wgt.tile([8, H], fp16, name="vT1", tag="vT1")
        v1a = scr.tile([P, 32], f32, name="v1a", tag="v1a")
        mul(v1a, e1n, sm("md1n"))
        v1b = scr.tile([P, 32], f32, name="v1b", tag="v1b")
        mul(v1b, sm("w1n"), sm("mo1n"))
        tt(v1n, v1a, v1b, op=ALU.add)
        for c in range(4):
            sq1 = scr.tile([P, 8], f32, name=f"sq1_{c}", tag="sq1")
            stt(sq1, v1n[:, 8 * c:8 * c + 8], 0, v1n[:, 8 * c:8 * c + 8],
                op0=ALU.bypass, op1=ALU.mult, accum_out=n1[:, c:c + 1])
        s1 = make_scale(n1, eg[1], (P, 4), "l1")
        ptall = psN.tile([8, H], f32, name="ptall", tag="pn")
        for c in range(4):
            nc.tensor.transpose(ptall[:, P * c:P * c + P],
                                v1n[:, 8 * c:8 * c + 8], ident)
            cp(vT1[:, P * c:P * c + P], ptall[:, P * c:P * c + P])

        s2 = norms(2, True)

        # ================= layer 1 batch ==================================
        h1 = wgt.tile([P, 4 * BC], fp16, name="h1", tag="h1")
        z1 = psA.tile([P, 4 * BC], f32, name="z1", tag="ps")
        for c in range(4):
            mm(z1[:, BC * c:BC * c + BC], vT1[:, P * c:P * c + P], xT)
            act(h1[:, BC * c:BC * c + BC], z1[:, BC * c:BC * c + BC], E.Tanh,
                bias=sm("b1")[:, c:c + 1], scale=s1[:, c:c + 1])
        hq1 = scr.tile([P, 4 * BC], fp16, name="hq1", tag="hq")
        gtt(hq1, h1, h1, op=ALU.mult)
        sc1 = scr.tile([P, 4 * BC], fp16, name="sc1", tag="sech2")
        ts(sc1, hq1, -1.0, 1.0, op0=ALU.mult, op1=ALU.add)
        F1 = wgt.tile([P, 4 * BC], bf16, name="F1", tag="F1")
        for c in range(4):
            ts(F1[:, BC * c:BC * c + BC], sc1[:, BC * c:BC * c + BC],
               e1d[:, c:c + 1], None, op0=ALU.mult)

        wdG2 = make_wdG(2, s1)

        # ================= layer 2/3 batch ================================
        def big_batch(l, s, wdG, h_prev, F_prev, mid=None):
            base = 0 if l == 2 else L3OFF
            hl = wgt.tile([P, 4 * BC], fp16, name=f"h{l}", tag=f"h{l}")
            zh = psA.tile([P, 4 * BC], f32, name=f"zh{l}", tag="ps")
            zg = psA.tile([P, 4 * BC], f32, name=f"zg{l}", tag="ps")
            for c in range(4):
                for k in range(c + 1):
                    mm(zh[:, BC * c:BC * c + BC],
                       vt[:, base + H * k + P * c:base + H * k + P * c + P],
                       h_prev[:, BC * k:BC * k + BC],
                       start=(k == 0), stop=(k == c))
                act(hl[:, BC * c:BC * c + BC], zh[:, BC * c:BC * c + BC], E.Tanh,
                    bias=sm(f"b{l}")[:, c:c + 1], scale=s[:, c:c + 1])
                mm(zg[:, BC * c:BC * c + BC], wdG[:, P * c:P * c + P],
                   F_prev[:, BC * c:BC * c + BC], start=True, stop=True)
                if mid is not None and c == 1:
                    mid()
            hql = scr.tile([P, 4 * BC], fp16, name=f"hq{l}", tag="hq")
            gtt(hql, hl, hl, op=ALU.mult)
            scl = scr.tile([P, 4 * BC], fp16, name=f"sc{l}", tag="sech2")
            ts(scl, hql, -1.0, 1.0, op0=ALU.mult, op1=ALU.add)
            Fl = wgt.tile([P, 4 * BC], bf16, name=f"F{l}", tag=f"F{l}")
            tt(Fl, scl, zg, op=ALU.mult)
            return hl, Fl

        # L3 norm chain emitted mid-L2 so its PE/Sc work lands between
        # L2 chunks (scheduler priority order follows emission order).
        s3_box = {}

        def l3_norms():
            s3_box["s"] = norms(3, False)

        h2, F2 = big_batch(2, s2, wdG2, h1, F1, mid=l3_norms)
        s3 = s3_box["s"]
        wdG3 = make_wdG(3, s2)

        # ================= layer 4 prep (transposed [512,8]) ==============
        vt4 = wgt.tile([P, 32], fp16, name="vt4", tag="vt4")
        v4a = scr.tile([P, 32], f32, name="v4a", tag="v4a")
        mul(v4a, e4t, sm("md4t"))
        v4b = scr.tile([P, 32], f32, name="v4b", tag="v4b")
        mul(v4b, sm("w4t"), sm("mo4t"))
        tt(vt4, v4a, v4b, op=ALU.add)
        vsq4 = scr.tile([P, 32], fp16, name="vsq4", tag="vsq4")
        act(vsq4, vt4, E.Square)
        pn4 = psN.tile([8, 2], f32, name="pn4", tag="pn")
        for k in range(4):
            mm(pn4, vsq4[:, 8 * k:8 * k + 8], ones2, start=(k == 0), stop=(k == 3))
        s4 = make_scale(pn4[:, 0:1], eg4, (8, 1), "l4")

        h3, F3 = big_batch(3, s3, wdG3, h2, F2)

        vd4 = wgt.tile([P, 32], bf16, name="vd4", tag="vd4")
        for k in range(4):
            ts(vd4[:, 8 * k:8 * k + 8], v4a[:, 8 * k:8 * k + 8],
               s3[:, k:k + 1], None, op0=ALU.mult)

        # ================= layer 4 batch ==================================
        z4 = psA.tile([P, 4 * BC], f32, name="z4", tag="ps")
        pz4 = z4[0:8, 0:BC]
        pf4 = z4[0:8, BC:2 * BC]
        for k in range(4):
            mm(pz4, vt4[:, 8 * k:8 * k + 8], h3[:, BC * k:BC * k + BC],
               start=(k == 0), stop=(k == 3))
        h4 = wgt.tile([8, BC], f32, name="h4", tag="h4")
        act(h4, pz4, E.Tanh, bias=b4, scale=s4)
        nc.sync.dma_start(t["hT_out"], h4)
        for k in range(4):
            mm(pf4, vd4[:, 8 * k:8 * k + 8], F3[:, BC * k:BC * k + BC],
               start=(k == 0), stop=(k == 3))
        hq4 = scr.tile([8, BC], f32, name="hq4", tag="hq4")
        mul(hq4, h4, h4)
        s24 = scr.tile([8, BC], f32, name="s24", tag="s24")
        ts(s24, hq4, -1.0, 1.0, op0=ALU.mult, op1=ALU.add)
        gt = scr.tile([8, BC], f32, name="gt", tag="gt")
        stt(gt, pf4, s4, s24, op0=ALU.mult, op1=ALU.mult)
        # fast natural log: ln(x) ~= A*bits(x) + B   (max abs err ~0.03)
        gbits = scr.tile([8, BC], f32, name="gbits", tag="gbits")
        cp(gbits, gt.bitcast(i32))
        sld = wgt.tile([8, BC], f32, name="sld", tag="sld")
        ts(sld, gbits, FASTLN_A, FASTLN_B, op0=ALU.mult, op1=ALU.add)
        nc.gpsimd.dma_start(t["sldT_out"], sld)

    nc.compile()
    return nc


def _host_prep(x, W1, logg1, bias1, W2, logg2, bias2, W3, logg3, bias3,
               W4, logg4, bias4):
    """Pure layout prep (transpose / reshape / gather / masks / casts)."""
    f = np.float32

    def cols(a):          # [512]-ish vector -> [128, 4] column-chunk layout
        return np.ascontiguousarray(np.reshape(a, (4, P)).T).astype(f)

    smalls = np.zeros((P, SMALL_W), f)

    def put(name, arr):
        a, b = _SM[name]
        smalls[:arr.shape[0], a:b] = arr

    def fold(m):          # [512, 8] -> [128, (k x)] with k = row-chunk
        return m.reshape(4, P, 8).transpose(1, 0, 2).reshape(P, 32)

    put("ident", np.eye(P, dtype=f))
    put("w1n", fold(W1))                                   # natural [512,8]
    put("w4t", fold(np.ascontiguousarray(W4.T)))           # [512,8]
    put("w1dg", cols(W1[np.arange(H), np.arange(H) // 64]))
    put("w4dg", cols(W4[np.arange(H) // 64, np.arange(H)]))
    put("lg1", cols(logg1)); put("b1", cols(bias1))
    put("lg2", cols(logg2)); put("b2", cols(bias2))
    put("lg3", cols(logg3)); put("b3", cols(bias3))
    put("lg4c", logg4.reshape(8, 1).astype(f))
    put("b4c", bias4.reshape(8, 1).astype(f))
    # structural masks
    o = np.arange(H)[:, None] // 64
    i1 = np.arange(8)[None, :]
    md1 = (i1 == o).astype(f); mo1 = (i1 < o).astype(f)    # [512, 8] natural
    put("md1n", fold(md1)); put("mo1n", fold(mo1))
    ii = np.arange(H)[:, None] // 64
    o4 = np.arange(8)[None, :]
    md4 = (o4 == ii).astype(f); mo4 = (o4 > ii).astype(f)  # [512, 8] transposed
    put("md4t", fold(md4)); put("mo4t", fold(mo4))

    w2T = np.ascontiguousarray(W2.T).astype(np.float16)
    w3T = np.ascontiguousarray(W3.T).astype(np.float16)
    xT = np.ascontiguousarray(x.T).astype(np.float16)      # [8, 2048]
    return xT, w2T, w3T, smalls


def kernel(**inputs):
    global LAST_RESULTS
    from concourse.bass_utils import run_bass_kernel_spmd

    xT, w2T, w3T, smalls = _host_prep(**{k: np.asarray(v) for k, v in inputs.items()})

    if "nc" not in _CACHE:
        _CACHE["nc"] = _build()
    nc = _CACHE["nc"]

    in_maps = []
    for c in range(NCORE):
        in_maps.append({
            "xT": np.ascontiguousarray(xT[:, BC * c:BC * (c + 1)]),
            "w2T": w2T, "w3T": w3T, "smalls": smalls,
        })
    res = run_bass_kernel_spmd(nc, in_maps, core_ids=list(range(NCORE)),
                               trace=TRACE)
    LAST_RESULTS = res

    B = BC * NCORE
    h = np.empty((B, 8), np.float32)
    sld = np.empty((B, 8), np.float32)
    for c, r in enumerate(res.results):
        h[BC * c:BC * (c + 1)] = r["hT_out"].T
        sld[BC * c:BC * (c + 1)] = r["sldT_out"].T
    return h, sld


# revision 14
# speedup vs baseline: 1.3867x; 1.1149x over previous
"""BNAF forward (B=2048, D=8, H=512, 4 masked layers) on 8 TRN2 NeuronCores.

Strategy
--------
Pure data parallel: batch is split 256/core; the small weights are replicated.

Math: the BNAF log-det recursion collapses in exp space: exp(logdet diag
blocks) == diag blocks of the normalized weight, exp(tanh logdet) == 1-h^2,
so the flow is a chain of positive block-diag matmuls with one log at the
end (2-op DVE fast-log).  The norm scale s=exp(logg)/||v|| is folded
input-side into the next layer's G-flow weights (wd form).  The sech^2
factor is applied as (h^2-1) -- the sign flips cancel across the even
number of layers (with the matching (h4^2-1) fold at L4).

Tile layout notes (trn2 Tile framework tracks deps at TILE granularity, so
false-sharing serializes):
- per-(layer,chunk) PSUM tiles for the h-path matmuls so chunk c+1's MMs
  don't wait on chunk c's tanh (whole-tile WAR).
- weights live in vtO (full rows, DMA-only writers) + vtD (the 4 diag
  128-blocks, strided DMA) per layer, so norm/exp work on vtD never blocks
  reads of the raw off-diag blocks.
- norm^2 columnize is 10 direct lhsT=v^2-window matmuls (no row-sum /
  transpose machinery); one-step Newton rsqrt from a magic seed.
"""

import numpy as np

TRACE = False          # set by test.py for profiling runs
LAST_RESULTS = None    # BassKernelResults stash for test.py

_CACHE = {}

P = 128
BC = 256          # batch per core
H = 512
NCORE = 8
MAGIC = 0x5f3759df

# smalls layout: first the exp-batch block (one ACT op), then the rest
_SM = {}
_off = 0
for _name, _w in [("w1dg", 4), ("w4dg", 4), ("lg1", 4), ("lg2", 4),
                  ("lg3", 4), ("lg4c", 1), ("w1n", 32), ("w4t", 32),  # exp blk
                  ("b4c", 1), ("b1", 4), ("b2", 4), ("b3", 4),
                  ("ident", 128),
                  ("md1n", 32), ("mo1n", 32), ("md4t", 32), ("mo4t", 32)]:
    _SM[_name] = (_off, _off + _w)
    _off += _w
SMALL_W = _off
EXPW = _SM["w4t"][1]           # width of the exp block (85)

FASTLN_A = float(np.log(2.0) / (1 << 23))
FASTLN_B = float((0.0430 - 127.0) * np.log(2.0))


def _build():
    import concourse.bacc as bacc
    import concourse.mybir as mybir
    import concourse.tile as tile
    from concourse.bass import AP
    from contextlib import ExitStack

    f32 = mybir.dt.float32
    u32 = mybir.dt.uint32
    i32 = mybir.dt.int32
    bf16 = mybir.dt.bfloat16
    fp16 = mybir.dt.float16
    E = mybir.ActivationFunctionType
    ALU = mybir.AluOpType

    nc = bacc.Bacc("TRN2", target_bir_lowering=False, debug=False,
                   enable_asserts=False, num_devices=NCORE)

    t = {}
    t["xT"] = nc.dram_tensor("xT", (8, BC), fp16, kind="ExternalInput").ap()
    t["w2T"] = nc.dram_tensor("w2T", (H, H), fp16, kind="ExternalInput").ap()
    t["w3T"] = nc.dram_tensor("w3T", (H, H), fp16, kind="ExternalInput").ap()
    t["smalls"] = nc.dram_tensor("smalls", (P, SMALL_W), f32, kind="ExternalInput").ap()
    t["hT_out"] = nc.dram_tensor("hT_out", (8, BC), f32, kind="ExternalOutput").ap()
    t["sldT_out"] = nc.dram_tensor("sldT_out", (8, BC), f32, kind="ExternalOutput").ap()

    def mm(out, lhsT, rhs, **kw):
        nc.tensor.matmul(out, lhsT, rhs, **kw)

    def winap(base_tile, p0, np_, col0, n, stride, w):
        """[np_ parts at p0] x (n windows of w cols, stride apart, from col0)."""
        base = base_tile[p0:p0 + np_, col0:col0 + w]
        return AP(base.tensor, base.offset,
                  [[base.ap[0][0], np_], [stride, n], [1, w]])

    with tile.TileContext(nc) as tc, ExitStack() as ctx:
        wgt = ctx.enter_context(tc.tile_pool(name="wgt", bufs=1))
        scr = ctx.enter_context(tc.tile_pool(name="scr", bufs=4))
        psN = ctx.enter_context(tc.tile_pool(name="psN", bufs=2, space="PSUM"))
        pzc = ctx.enter_context(tc.tile_pool(name="pzc", bufs=4, space="PSUM"))
        pzg = ctx.enter_context(tc.tile_pool(name="pzg", bufs=1, space="PSUM"))

        act = nc.scalar.activation
        cp = nc.vector.tensor_copy
        ts = nc.vector.tensor_scalar
        stt = nc.vector.scalar_tensor_tensor
        mul = nc.vector.tensor_mul
        tt = nc.vector.tensor_tensor
        gtt = nc.gpsimd.tensor_tensor
        gms = nc.gpsimd.memset

        # ---- dummy ACT at t0 pulls the single exp_and_others table load ---
        dmy = wgt.tile([P, 1], f32, name="dmy", tag="dmy")
        dmyo = wgt.tile([P, 1], f32, name="dmyo", tag="dmyo")
        nc.vector.memset(dmy, 0.0)
        act(dmyo, dmy, E.Exp)

        # ---- input DMAs ---------------------------------------------------
        # vtO: full rows (raw W, DMA is the only writer).  vtD: the four
        # diag 128-blocks per layer, chunk c at cols [128c, 128c+128).
        smalls = wgt.tile([P, SMALL_W], f32, name="smalls_t", tag="smalls_t")
        xT = wgt.tile([8, BC], fp16, name="xT_t", tag="xT_t")
        vtO = {l: wgt.tile([P, 4 * H], fp16, name=f"vtO{l}", tag=f"vtO{l}")
               for l in (2, 3)}
        vtD = {l: wgt.tile([P, 4 * P], fp16, name=f"vtD{l}", tag=f"vtD{l}")
               for l in (2, 3)}
        nc.sync.dma_start(smalls, t["smalls"])
        nc.gpsimd.dma_start(vtD[2], AP(t["w2T"].tensor, 0,
                                       [[H, P], [P * H + P, 4], [1, P]]))
        nc.gpsimd.dma_start(vtD[3], AP(t["w3T"].tensor, 0,
                                       [[H, P], [P * H + P, 4], [1, P]]))
        for hh in (0, 1):
            src = AP(t["w2T"].tensor, hh * 2 * P * H,
                     [[H, P], [P * H, 2], [1, H]])
            nc.sync.dma_start(vtO[2][:, 2 * H * hh:2 * H * hh + 2 * H], src)
        nc.sync.dma_start(xT, t["xT"])
        for hh in (0, 1):
            src = AP(t["w3T"].tensor, hh * 2 * P * H,
                     [[H, P], [P * H, 2], [1, H]])
            nc.gpsimd.dma_start(vtO[3][:, 2 * H * hh:2 * H * hh + 2 * H], src)

        def sm(name):
            a, b = _SM[name]
            return smalls[:, a:b]

        ident = sm("ident")
        b4 = smalls[0:8, _SM["b4c"][0]:_SM["b4c"][1]]

        # ---- constants ----------------------------------------------------
        ones2f = wgt.tile([P, 2], f32, name="ones2f", tag="ones2f")
        ones2 = wgt.tile([P, 2], fp16, name="ones2", tag="ones2")
        magict = wgt.tile([P, 5], u32, name="magict", tag="magict")
        gms(ones2f, 1.0)
        gms(magict, MAGIC)
        cp(ones2, ones2f)

        # ---- diag prep: zero LL quadrants, exp the 64-blocks --------------
        for l in (2, 3):
            gms(winap(vtD[l], 64, 64, 0, 4, P, 64), 0.0)
        for l in (2, 3):
            dA = winap(vtD[l], 0, 64, 0, 4, P, 64)
            dB = winap(vtD[l], 64, 64, 64, 4, P, 64)
            act(dA, dA, E.Exp)
            act(dB, dB, E.Exp)

        # ---- one exp over all the small weight pieces ---------------------
        esm = wgt.tile([P, EXPW], f32, name="esm", tag="esm")
        act(esm, smalls[:, 0:EXPW], E.Exp)
        e1n = esm[:, _SM["w1n"][0]:_SM["w1n"][1]]
        e4t = esm[:, _SM["w4t"][0]:_SM["w4t"][1]]
        e1d = esm[:, _SM["w1dg"][0]:_SM["w1dg"][1]]
        eg = {1: esm[:, _SM["lg1"][0]:_SM["lg1"][1]],
              2: esm[:, _SM["lg2"][0]:_SM["lg2"][1]]}
        eg34 = esm[:, _SM["lg3"][0]:_SM["lg4c"][1]]   # [128, 5]: lg3 | lg4c

        # s = eg * rsqrt(n2): magic seed + one Newton step
        def make_scale(n2_ap, eg_ap, shape, nm):
            pr = shape[0]
            n2s = scr.tile(list(shape), f32, name=f"n2s_{nm}", tag="sc_n2s")
            cp(n2s, n2_ap)
            shf = scr.tile(list(shape), u32, name=f"shf_{nm}", tag="sc_shf")
            ts(shf, n2s.bitcast(u32), 1, None, op0=ALU.arith_shift_right)
            y0 = scr.tile(list(shape), u32, name=f"y0_{nm}", tag="sc_y0")
            stt(y0, magict[:pr, :shape[1]], 0, shf, op0=ALU.bypass, op1=ALU.subtract)
            y = y0.bitcast(f32)
            t1 = scr.tile(list(shape), f32, name=f"t1_{nm}", tag="sc_t1")
            t2 = scr.tile(list(shape), f32, name=f"t2_{nm}", tag="sc_t2")
            mul(t1, y, y)
            mul(t2, t1, n2s)
            ts(t1, t2, -0.5, 1.5, op0=ALU.mult, op1=ALU.add)
            yn = scr.tile(list(shape), f32, name=f"yn_{nm}", tag="sc_yn")
            mul(yn, y, t1)
            s = wgt.tile(list(shape), f32, name=f"s_{nm}", tag=f"s_{nm}")
            mul(s, eg_ap, yn)
            return s

        # ---- v^2 pieces + direct columnized norm matmuls ------------------
        vsqO = {l: scr.tile([P, 4 * H], fp16, name=f"vsqO{l}", tag=f"vsqO{l}")
                for l in (2, 3)}
        vsqD = {l: scr.tile([P, 4 * P], fp16, name=f"vsqD{l}", tag=f"vsqD{l}")
                for l in (2, 3)}

        def vsq_off(l, eng):
            for k in range(3):
                a, b = H * k + P * (k + 1), H * (k + 1)
                eng(vsqO[l][:, a:b], vtO[l][:, a:b], vtO[l][:, a:b], op=ALU.mult)

        def vsq_diag(l):
            tt(vsqD[l], vtD[l], vtD[l], op=ALU.mult)

        def ncol_direct(l):
            ncol = psN.tile([P, 4], f32, name=f"ncol{l}", tag="pn")
            # off-window contributions first (DMA-gated only), then diag.
            # Only the first-executed MM clears the bank (start=True): all
            # later MMs write-or-accumulate per element via has_written.
            for c in range(1, 4):
                for k in range(c):
                    mm(ncol[:, c:c + 1], vsqO[l][:, H * k + P * c:H * k + P * c + P],
                       ones2[:, 0:1], start=(c == 1 and k == 0), stop=False,
                       skip_group_check=True)
            for c in range(4):
                mm(ncol[:, c:c + 1], vsqD[l][:, P * c:P * c + P], ones2[:, 0:1],
                   start=False, stop=True, skip_group_check=True)
            return ncol

        # wdG: [dA 0; 0 dB] with input-side s_prev fold (per-partition)
        def make_wdG(l, s_prev):
            wdG = wgt.tile([P, 4 * P], fp16, name=f"wdG{l}", tag=f"wdG{l}")
            for c in range(4):
                ts(wdG[:, P * c:P * c + P], vtD[l][:, P * c:P * c + P],
                   s_prev[:, c:c + 1], None, op0=ALU.mult)
            gms(winap(wdG, 0, 64, 64, 4, P, 64), 0.0)   # zero the X quadrant
            return wdG

        # ================= layer 1 prep (natural layout [512,8]) ==========
        v1n = wgt.tile([P, 32], f32, name="v1n", tag="v1n")
        n1 = scr.tile([P, 4], f32, name="n1", tag="n1")
        vT1 = wgt.tile([8, H], fp16, name="vT1", tag="vT1")
        v1a = scr.tile([P, 32], f32, name="v1a", tag="v1a")
        mul(v1a, e1n, sm("md1n"))
        v1b = scr.tile([P, 32], f32, name="v1b", tag="v1b")
        mul(v1b, sm("w1n"), sm("mo1n"))
        tt(v1n, v1a, v1b, op=ALU.add)
        for c in range(4):
            sq1 = scr.tile([P, 8], f32, name=f"sq1_{c}", tag="sq1")
            stt(sq1, v1n[:, 8 * c:8 * c + 8], 0, v1n[:, 8 * c:8 * c + 8],
                op0=ALU.bypass, op1=ALU.mult, accum_out=n1[:, c:c + 1])
        s1 = make_scale(n1, eg[1], (P, 4), "l1")
        ptall = psN.tile([8, H], f32, name="ptall", tag="pn")
        for c in range(4):
            nc.tensor.transpose(ptall[:, P * c:P * c + P],
                                v1n[:, 8 * c:8 * c + 8], ident)
        act(vT1, ptall, E.Copy)     # one PSUM->SBUF copy on Scalar

        # L2 norm chain (s2) -- as early as possible
        vsq_off(2, gtt)
        vsq_diag(2)
        ncol2 = ncol_direct(2)
        s2 = make_scale(ncol2, eg[2], (P, 4), "l2")

        # ================= layer 1 batch ==================================
        h1 = wgt.tile([P, 4 * BC], fp16, name="h1", tag="h1")
        for c in range(4):
            z = pzc.tile([P, BC], f32, name=f"z1_{c}", tag="ps")
            mm(z, vT1[:, P * c:P * c + P], xT)
            act(h1[:, BC * c:BC * c + BC], z, E.Tanh,
                bias=sm("b1")[:, c:c + 1], scale=s1[:, c:c + 1])
        wdG2 = make_wdG(2, s1)
        # D1 = -F1 = (h1^2 - 1) * e1d  (sign flips cancel across layers)
        hq1 = scr.tile([P, 4 * BC], fp16, name="hq1", tag="hq1")
        gtt(hq1, h1, h1, op=ALU.mult)
        sc1 = scr.tile([P, 4 * BC], fp16, name="sc1", tag="sech2")
        ts(sc1, hq1, 1.0, 1.0, op0=ALU.mult, op1=ALU.subtract)   # h^2 - 1
        D1 = wgt.tile([P, 4 * BC], bf16, name="D1", tag="D1")
        for c in range(4):
            ts(D1[:, BC * c:BC * c + BC], sc1[:, BC * c:BC * c + BC],
               e1d[:, c:c + 1], None, op0=ALU.mult)

        # L3 norm pieces (start early; consumed mid-L2)
        vsq_off(3, gtt)
        vsq_diag(3)

        # ================= layer 2/3 batch ================================
        def big_batch(l, s, wdG, h_prev, D_prev, mid=None):
            hl = wgt.tile([P, 4 * BC], fp16, name=f"h{l}", tag=f"h{l}")
            zg = pzg.tile([P, 4 * BC], f32, name=f"zg{l}", tag="psg")
            for c in range(4):
                z = pzc.tile([P, BC], f32, name=f"zh{l}_{c}", tag="ps")
                for k in range(c + 1):
                    lhsT = (vtD[l][:, P * c:P * c + P] if k == c else
                            vtO[l][:, H * k + P * c:H * k + P * c + P])
                    mm(z, lhsT, h_prev[:, BC * k:BC * k + BC],
                       start=(k == 0), stop=(k == c))
                act(hl[:, BC * c:BC * c + BC], z, E.Tanh,
                    bias=sm(f"b{l}")[:, c:c + 1], scale=s[:, c:c + 1])
                mm(zg[:, BC * c:BC * c + BC], wdG[:, P * c:P * c + P],
                   D_prev[:, BC * c:BC * c + BC], start=True, stop=True)
                if mid is not None and c == 1:
                    mid()
            hql = scr.tile([P, 4 * BC], fp16, name=f"hq{l}", tag=f"hq{l}")
            gtt(hql, hl, hl, op=ALU.mult)
            Dl = wgt.tile([P, 4 * BC], bf16, name=f"D{l}", tag=f"D{l}")
            # D_l = (h^2 - 1) * zg  (zg = wdG @ D_{l-1}, PSUM fp32)
            stt(Dl, hql, 1.0, zg, op0=ALU.subtract, op1=ALU.mult)
            return hl, Dl

        # s3+s4 batched scale chain, emitted mid-L2 via callback
        s34_box = {}

        def mid_l2():
            ncol3 = ncol_direct(3)
            # layer-4 prep
            v4a = scr.tile([P, 32], f32, name="v4a", tag="v4a")
            mul(v4a, e4t, sm("md4t"))
            v4b = scr.tile([P, 32], f32, name="v4b", tag="v4b")
            mul(v4b, sm("w4t"), sm("mo4t"))
            vt4 = wgt.tile([P, 32], fp16, name="vt4", tag="vt4")
            tt(vt4, v4a, v4b, op=ALU.add)
            vsq4 = scr.tile([P, 32], fp16, name="vsq4", tag="vsq4")
            act(vsq4, vt4, E.Square)
            pn4 = psN.tile([8, 2], f32, name="pn4", tag="pn")
            for k in range(4):
                mm(pn4, vsq4[:, 8 * k:8 * k + 8], ones2,
                   start=(k == 0), stop=(k == 3))
            n34 = scr.tile([P, 5], f32, name="n34", tag="n34")
            nc.vector.memset(n34[:, 4:5], 1.0)   # rows 8.. of the s4 col
            cp(n34[:, 0:4], ncol3)
            cp(n34[0:8, 4:5], pn4[:, 0:1])
            s34 = make_scale(n34, eg34, (P, 5), "l34")
            s34_box["s3"] = s34[:, 0:4]
            s34_box["s4"] = s34[0:8, 4:5]
            s34_box["v4a"] = v4a
            s34_box["vt4"] = vt4

        h2, D2 = big_batch(2, s2, wdG2, h1, D1, mid=mid_l2)
        s3, s4 = s34_box["s3"], s34_box["s4"]
        v4a, vt4 = s34_box["v4a"], s34_box["vt4"]
        wdG3 = make_wdG(3, s2)

        h3, D3 = big_batch(3, s3, wdG3, h2, D2)

        vd4 = wgt.tile([P, 32], bf16, name="vd4", tag="vd4")
        for k in range(4):
            ts(vd4[:, 8 * k:8 * k + 8], v4a[:, 8 * k:8 * k + 8],
               s3[:, k:k + 1], None, op0=ALU.mult)

        # ================= layer 4 batch ==================================
        pz4 = pzc.tile([8, BC], f32, name="pz4", tag="ps")
        for k in range(4):
            mm(pz4, vt4[:, 8 * k:8 * k + 8], h3[:, BC * k:BC * k + BC],
               start=(k == 0), stop=(k == 3))
        h4 = wgt.tile([8, BC], f32, name="h4", tag="h4")
        act(h4, pz4, E.Tanh, bias=b4, scale=s4)
        nc.sync.dma_start(t["hT_out"], h4)
        pf4 = pzc.tile([8, BC], f32, name="pf4", tag="ps")
        for k in range(4):
            mm(pf4, vd4[:, 8 * k:8 * k + 8], D3[:, BC * k:BC * k + BC],
               start=(k == 0), stop=(k == 3))
        hq4 = scr.tile([8, BC], f32, name="hq4", tag="hq4")
        mul(hq4, h4, h4)
        s24 = scr.tile([8, BC], f32, name="s24", tag="s24")
        ts(s24, hq4, 1.0, 1.0, op0=ALU.mult, op1=ALU.subtract)   # h4^2 - 1
        gt = scr.tile([8, BC], f32, name="gt", tag="gt")
        stt(gt, pf4, s4, s24, op0=ALU.mult, op1=ALU.mult)
        # fast natural log: ln(x) ~= A*bits(x) + B   (max abs err ~0.03)
        gbits = scr.tile([8, BC], f32, name="gbits", tag="gbits")
        cp(gbits, gt.bitcast(i32))
        sld = wgt.tile([8, BC], f32, name="sld", tag="sld")
        ts(sld, gbits, FASTLN_A, FASTLN_B, op0=ALU.mult, op1=ALU.add)
        nc.gpsimd.dma_start(t["sldT_out"], sld)

    nc.compile()
    return nc


def _host_prep(x, W1, logg1, bias1, W2, logg2, bias2, W3, logg3, bias3,
               W4, logg4, bias4):
    """Pure layout prep (transpose / reshape / gather / masks / casts)."""
    f = np.float32

    def cols(a):          # [512]-ish vector -> [128, 4] column-chunk layout
        return np.ascontiguousarray(np.reshape(a, (4, P)).T).astype(f)

    smalls = np.zeros((P, SMALL_W), f)

    def put(name, arr):
        a, b = _SM[name]
        smalls[:arr.shape[0], a:b] = arr

    def fold(m):          # [512, 8] -> [128, (k x)] with k = row-chunk
        return m.reshape(4, P, 8).transpose(1, 0, 2).reshape(P, 32)

    put("ident", np.eye(P, dtype=f))
    put("w1n", fold(W1))                                   # natural [512,8]
    put("w4t", fold(np.ascontiguousarray(W4.T)))           # [512,8]
    put("w1dg", cols(W1[np.arange(H), np.arange(H) // 64]))
    put("w4dg", cols(W4[np.arange(H) // 64, np.arange(H)]))
    put("lg1", cols(logg1)); put("b1", cols(bias1))
    put("lg2", cols(logg2)); put("b2", cols(bias2))
    put("lg3", cols(logg3)); put("b3", cols(bias3))
    put("lg4c", logg4.reshape(8, 1).astype(f))
    put("b4c", bias4.reshape(8, 1).astype(f))
    # structural masks
    o = np.arange(H)[:, None] // 64
    i1 = np.arange(8)[None, :]
    md1 = (i1 == o).astype(f); mo1 = (i1 < o).astype(f)    # [512, 8] natural
    put("md1n", fold(md1)); put("mo1n", fold(mo1))
    ii = np.arange(H)[:, None] // 64
    o4 = np.arange(8)[None, :]
    md4 = (o4 == ii).astype(f); mo4 = (o4 > ii).astype(f)  # [512, 8] transposed
    put("md4t", fold(md4)); put("mo4t", fold(mo4))

    w2T = np.ascontiguousarray(W2.T).astype(np.float16)
    w3T = np.ascontiguousarray(W3.T).astype(np.float16)
    xT = np.ascontiguousarray(x.T).astype(np.float16)      # [8, 2048]
    return xT, w2T, w3T, smalls


def kernel(**inputs):
    global LAST_RESULTS
    from concourse.bass_utils import run_bass_kernel_spmd

    xT, w2T, w3T, smalls = _host_prep(**{k: np.asarray(v) for k, v in inputs.items()})

    if "nc" not in _CACHE:
        _CACHE["nc"] = _build()
    nc = _CACHE["nc"]

    in_maps = []
    for c in range(NCORE):
        in_maps.append({
            "xT": np.ascontiguousarray(xT[:, BC * c:BC * (c + 1)]),
            "w2T": w2T, "w3T": w3T, "smalls": smalls,
        })
    res = run_bass_kernel_spmd(nc, in_maps, core_ids=list(range(NCORE)),
                               trace=TRACE)
    LAST_RESULTS = res

    B = BC * NCORE
    h = np.empty((B, 8), np.float32)
    sld = np.empty((B, 8), np.float32)
    for c, r in enumerate(res.results):
        h[BC * c:BC * (c + 1)] = r["hT_out"].T
        sld[BC * c:BC * (c + 1)] = r["sldT_out"].T
    return h, sld


# revision 20
# speedup vs baseline: 1.5348x; 1.1068x over previous
"""BNAF forward (B=2048, D=8, H=512, 4 masked layers) on 8 TRN2 NeuronCores.

Strategy
--------
Pure data parallel: batch is split 256/core; the small weights are replicated.

Math: the BNAF log-det recursion collapses in exp space: exp(logdet diag
blocks) == diag blocks of the normalized weight, exp(tanh logdet) == 1-h^2,
so the flow is a chain of positive block-diag matmuls with one log at the
end (2-op DVE fast-log).  The norm scale s=exp(logg)/||v|| is folded
input-side into the next layer's G-flow weights (wd form).  The sech^2
factor is applied as (h^2-1) -- the sign flips cancel across the even
number of layers (with the matching (h4^2-1) fold at L4).

Tile layout notes (trn2 Tile framework tracks deps at TILE granularity, so
false-sharing serializes):
- per-(layer,chunk) PSUM tiles for the h-path matmuls so chunk c+1's MMs
  don't wait on chunk c's tanh (whole-tile WAR).
- weights live in vtO (full rows, DMA-only writers) + vtD (the 4 diag
  128-blocks, strided DMA) per layer, so norm/exp work on vtD never blocks
  reads of the raw off-diag blocks.
- norm^2 columnize is 10 direct lhsT=v^2-window matmuls (no row-sum /
  transpose machinery); one-step Newton rsqrt from a magic seed.
"""

import numpy as np

TRACE = False          # set by test.py for profiling runs
LAST_RESULTS = None    # BassKernelResults stash for test.py

_CACHE = {}

P = 128
BC = 256          # batch per core
H = 512
NCORE = 8
MAGIC = 0x5f3759df

# smalls layout: first the exp-batch block (one ACT op), then the rest
_SM = {}
_off = 0
for _name, _w in [("w1dg", 4), ("w4dg", 4), ("lg1", 4), ("lg2", 4),
                  ("lg3", 4), ("lg4c", 1), ("w1n", 32), ("w4t", 32),  # exp blk
                  ("b4c", 1), ("b1", 4), ("b2", 4), ("b3", 4),
                  ("ident", 128),
                  ("md1n", 32), ("mo1n", 32), ("md4t", 32), ("mo4t", 32)]:
    _SM[_name] = (_off, _off + _w)
    _off += _w
SMALL_W = _off
EXPW = _SM["w4t"][1]           # width of the exp block (85)

FASTLN_A = float(np.log(2.0) / (1 << 23))
FASTLN_B = float((0.0430 - 127.0) * np.log(2.0))


def _build():
    import concourse.bacc as bacc
    import concourse.mybir as mybir
    import concourse.tile as tile
    from concourse.bass import AP
    from contextlib import ExitStack

    f32 = mybir.dt.float32
    u32 = mybir.dt.uint32
    i32 = mybir.dt.int32
    bf16 = mybir.dt.bfloat16
    fp16 = mybir.dt.float16
    E = mybir.ActivationFunctionType
    ALU = mybir.AluOpType

    nc = bacc.Bacc("TRN2", target_bir_lowering=False, debug=False,
                   enable_asserts=False, num_devices=NCORE)

    t = {}
    t["xT"] = nc.dram_tensor("xT", (8, BC), fp16, kind="ExternalInput").ap()
    t["w2T"] = nc.dram_tensor("w2T", (H, H), fp16, kind="ExternalInput").ap()
    t["w3T"] = nc.dram_tensor("w3T", (H, H), fp16, kind="ExternalInput").ap()
    t["smalls"] = nc.dram_tensor("smalls", (P, SMALL_W), f32, kind="ExternalInput").ap()
    t["hT_out"] = nc.dram_tensor("hT_out", (8, BC), f32, kind="ExternalOutput").ap()
    t["sldT_out"] = nc.dram_tensor("sldT_out", (8, BC), f32, kind="ExternalOutput").ap()

    def mm(out, lhsT, rhs, **kw):
        nc.tensor.matmul(out, lhsT, rhs, **kw)

    def winap(base_tile, p0, np_, col0, n, stride, w):
        """[np_ parts at p0] x (n windows of w cols, stride apart, from col0)."""
        base = base_tile[p0:p0 + np_, col0:col0 + w]
        return AP(base.tensor, base.offset,
                  [[base.ap[0][0], np_], [stride, n], [1, w]])

    with tile.TileContext(nc) as tc, ExitStack() as ctx:
        wgt = ctx.enter_context(tc.tile_pool(name="wgt", bufs=1))
        scr = ctx.enter_context(tc.tile_pool(name="scr", bufs=4))
        psN = ctx.enter_context(tc.tile_pool(name="psN", bufs=2, space="PSUM"))
        pzc = ctx.enter_context(tc.tile_pool(name="pzc", bufs=4, space="PSUM"))
        pzg = ctx.enter_context(tc.tile_pool(name="pzg", bufs=1, space="PSUM"))

        act = nc.scalar.activation
        cp = nc.vector.tensor_copy
        ts = nc.vector.tensor_scalar
        stt = nc.vector.scalar_tensor_tensor
        mul = nc.vector.tensor_mul
        tt = nc.vector.tensor_tensor
        gtt = nc.gpsimd.tensor_tensor
        gms = nc.gpsimd.memset

        # ---- dummy ACT at t0 pulls the single exp_and_others table load ---
        dmy = wgt.tile([P, 1], f32, name="dmy", tag="dmy")
        dmyo = wgt.tile([P, 1], f32, name="dmyo", tag="dmyo")
        nc.vector.memset(dmy, 0.0)
        act(dmyo, dmy, E.Exp)

        # ---- input DMAs ---------------------------------------------------
        # vtO: full rows (raw W, DMA is the only writer).  vtD: the four
        # diag 128-blocks per layer, chunk c at cols [128c, 128c+128).
        smalls = wgt.tile([P, SMALL_W], f32, name="smalls_t", tag="smalls_t")
        xT = wgt.tile([8, BC], fp16, name="xT_t", tag="xT_t")
        vtO = {l: wgt.tile([P, 4 * H], fp16, name=f"vtO{l}", tag=f"vtO{l}")
               for l in (2, 3)}
        vtD = {l: wgt.tile([P, 4 * P], fp16, name=f"vtD{l}", tag=f"vtD{l}")
               for l in (2, 3)}
        nc.sync.dma_start(smalls, t["smalls"])
        nc.gpsimd.dma_start(vtD[2], AP(t["w2T"].tensor, 0,
                                       [[H, P], [P * H + P, 4], [1, P]]))
        nc.gpsimd.dma_start(vtD[3], AP(t["w3T"].tensor, 0,
                                       [[H, P], [P * H + P, 4], [1, P]]))
        for hh in (0, 1):
            src = AP(t["w2T"].tensor, hh * 2 * P * H,
                     [[H, P], [P * H, 2], [1, H]])
            nc.sync.dma_start(vtO[2][:, 2 * H * hh:2 * H * hh + 2 * H], src)
        nc.sync.dma_start(xT, t["xT"])
        for hh in (0, 1):
            src = AP(t["w3T"].tensor, hh * 2 * P * H,
                     [[H, P], [P * H, 2], [1, H]])
            nc.gpsimd.dma_start(vtO[3][:, 2 * H * hh:2 * H * hh + 2 * H], src)

        def sm(name):
            a, b = _SM[name]
            return smalls[:, a:b]

        ident = sm("ident")
        b4 = smalls[0:8, _SM["b4c"][0]:_SM["b4c"][1]]

        # ---- constants ----------------------------------------------------
        ones2f = wgt.tile([P, 2], f32, name="ones2f", tag="ones2f")
        ones2 = wgt.tile([P, 2], fp16, name="ones2", tag="ones2")
        magict = wgt.tile([P, 5], u32, name="magict", tag="magict")
        gms(ones2f, 1.0)
        gms(magict, MAGIC)
        cp(ones2, ones2f)

        # ---- diag prep: zero LL quadrants, exp the 64-blocks --------------
        for l in (2, 3):
            gms(winap(vtD[l], 64, 64, 0, 4, P, 64), 0.0)
        for l in (2, 3):
            dA = winap(vtD[l], 0, 64, 0, 4, P, 64)
            dB = winap(vtD[l], 64, 64, 64, 4, P, 64)
            act(dA, dA, E.Exp)
            act(dB, dB, E.Exp)

        # ---- one exp over all the small weight pieces ---------------------
        esm = wgt.tile([P, EXPW], f32, name="esm", tag="esm")
        act(esm, smalls[:, 0:EXPW], E.Exp)
        e1n = esm[:, _SM["w1n"][0]:_SM["w1n"][1]]
        e4t = esm[:, _SM["w4t"][0]:_SM["w4t"][1]]
        e1d = esm[:, _SM["w1dg"][0]:_SM["w1dg"][1]]
        eg = {1: esm[:, _SM["lg1"][0]:_SM["lg1"][1]],
              2: esm[:, _SM["lg2"][0]:_SM["lg2"][1]]}
        eg34 = esm[:, _SM["lg3"][0]:_SM["lg4c"][1]]   # [128, 5]: lg3 | lg4c

        # s = eg * rsqrt(n2): magic seed + one Newton step.  The PSUM read
        # is on Vector; the chain arithmetic runs on GpSimd to keep the
        # Vector queue free for the batch-sized ops.
        gts = nc.gpsimd.tensor_scalar
        gstt = nc.gpsimd.scalar_tensor_tensor
        gmul = nc.gpsimd.tensor_mul

        def make_scale(n2_ap, eg_ap, shape, nm):
            pr = shape[0]
            n2s = scr.tile(list(shape), f32, name=f"n2s_{nm}", tag="sc_n2s")
            cp(n2s, n2_ap)
            shf = scr.tile(list(shape), u32, name=f"shf_{nm}", tag="sc_shf")
            ts(shf, n2s.bitcast(u32), 1, None, op0=ALU.arith_shift_right)
            y0 = scr.tile(list(shape), u32, name=f"y0_{nm}", tag="sc_y0")
            stt(y0, magict[:pr, :shape[1]], 0, shf, op0=ALU.bypass, op1=ALU.subtract)
            y = y0.bitcast(f32)
            t1 = scr.tile(list(shape), f32, name=f"t1_{nm}", tag="sc_t1")
            t2 = scr.tile(list(shape), f32, name=f"t2_{nm}", tag="sc_t2")
            gmul(t1, y, y)
            gmul(t2, t1, n2s)
            ts(t1, t2, -0.5, 1.5, op0=ALU.mult, op1=ALU.add)
            yn = scr.tile(list(shape), f32, name=f"yn_{nm}", tag="sc_yn")
            gmul(yn, y, t1)
            s = wgt.tile(list(shape), f32, name=f"s_{nm}", tag=f"s_{nm}")
            gmul(s, eg_ap, yn)
            return s

        # ---- v^2 pieces + direct columnized norm matmuls ------------------
        vsqO = {l: scr.tile([P, 4 * H], fp16, name=f"vsqO{l}", tag=f"vsqO{l}")
                for l in (2, 3)}
        vsqD = {l: scr.tile([P, 4 * P], fp16, name=f"vsqD{l}", tag=f"vsqD{l}")
                for l in (2, 3)}

        def vsq_off(l, eng):
            for k in range(3):
                a, b = H * k + P * (k + 1), H * (k + 1)
                eng(vsqO[l][:, a:b], vtO[l][:, a:b], vtO[l][:, a:b], op=ALU.mult)

        def vsq_diag(l):
            tt(vsqD[l], vtD[l], vtD[l], op=ALU.mult)

        def ncol_direct(l):
            ncol = psN.tile([P, 4], f32, name=f"ncol{l}", tag="pn")
            # off-window contributions first (DMA-gated only), then diag.
            # Only the first-executed MM clears the bank (start=True): all
            # later MMs write-or-accumulate per element via has_written.
            for c in range(1, 4):
                for k in range(c):
                    mm(ncol[:, c:c + 1], vsqO[l][:, H * k + P * c:H * k + P * c + P],
                       ones2[:, 0:1], start=(c == 1 and k == 0), stop=False,
                       skip_group_check=True)
            for c in range(4):
                mm(ncol[:, c:c + 1], vsqD[l][:, P * c:P * c + P], ones2[:, 0:1],
                   start=False, stop=True, skip_group_check=True)
            return ncol

        # wdG: [dA 0; 0 dB] with input-side s_prev fold (per-partition)
        def make_wdG(l, s_prev):
            wdG = wgt.tile([P, 4 * P], fp16, name=f"wdG{l}", tag=f"wdG{l}")
            for c in range(4):
                ts(wdG[:, P * c:P * c + P], vtD[l][:, P * c:P * c + P],
                   s_prev[:, c:c + 1], None, op0=ALU.mult)
            gms(winap(wdG, 0, 64, 64, 4, P, 64), 0.0)   # zero the X quadrant
            return wdG

        # ================= layer 1 prep (natural layout [512,8]) ==========
        v1n = wgt.tile([P, 32], f32, name="v1n", tag="v1n")
        n1 = scr.tile([P, 4], f32, name="n1", tag="n1")
        vT1 = wgt.tile([8, H], fp16, name="vT1", tag="vT1")
        v1a = scr.tile([P, 32], f32, name="v1a", tag="v1a")
        mul(v1a, e1n, sm("md1n"))
        v1b = scr.tile([P, 32], f32, name="v1b", tag="v1b")
        mul(v1b, sm("w1n"), sm("mo1n"))
        tt(v1n, v1a, v1b, op=ALU.add)
        for c in range(4):
            sq1 = scr.tile([P, 8], f32, name=f"sq1_{c}", tag="sq1")
            stt(sq1, v1n[:, 8 * c:8 * c + 8], 0, v1n[:, 8 * c:8 * c + 8],
                op0=ALU.bypass, op1=ALU.mult, accum_out=n1[:, c:c + 1])
        s1 = make_scale(n1, eg[1], (P, 4), "l1")
        ptall = psN.tile([8, H], f32, name="ptall", tag="pn")
        for c in range(4):
            nc.tensor.transpose(ptall[:, P * c:P * c + P],
                                v1n[:, 8 * c:8 * c + 8], ident)
        act(vT1, ptall, E.Copy)     # one PSUM->SBUF copy on Scalar

        # L2 norm chain (s2) -- as early as possible
        vsq_off(2, gtt)
        vsq_diag(2)
        ncol2 = ncol_direct(2)
        s2 = make_scale(ncol2, eg[2], (P, 4), "l2")

        # ================= layer 1 batch ==================================
        h1 = wgt.tile([P, 4 * BC], fp16, name="h1", tag="h1")
        for c in range(4):
            z = pzc.tile([P, BC], f32, name=f"z1_{c}", tag="ps")
            mm(z, vT1[:, P * c:P * c + P], xT)
            act(h1[:, BC * c:BC * c + BC], z, E.Tanh,
                bias=sm("b1")[:, c:c + 1], scale=s1[:, c:c + 1])
        wdG2 = make_wdG(2, s1)
        # D1 = -F1 = (h1^2 - 1) * e1d  (sign flips cancel across layers)
        hq1 = scr.tile([P, 4 * BC], fp16, name="hq1", tag="hq1")
        act(hq1, h1, E.Square)
        D1 = wgt.tile([P, 4 * BC], bf16, name="D1", tag="D1")
        for c in range(4):
            # (hq * e1d) - e1d == e1d * (h^2 - 1)
            ts(D1[:, BC * c:BC * c + BC], hq1[:, BC * c:BC * c + BC],
               e1d[:, c:c + 1], e1d[:, c:c + 1], op0=ALU.mult, op1=ALU.subtract)

        # L3 norm pieces (start early; consumed mid-L2)
        vsq_off(3, gtt)
        vsq_diag(3)

        # ================= layer 2/3 batch ================================
        def big_batch(l, s, wdG, h_prev, D_prev, mid=None):
            hl = wgt.tile([P, 4 * BC], fp16, name=f"h{l}", tag=f"h{l}")
            zg = pzg.tile([P, 4 * BC], f32, name=f"zg{l}", tag="psg")
            for c in range(4):
                z = pzc.tile([P, BC], f32, name=f"zh{l}_{c}", tag="ps")
                for k in range(c + 1):
                    lhsT = (vtD[l][:, P * c:P * c + P] if k == c else
                            vtO[l][:, H * k + P * c:H * k + P * c + P])
                    mm(z, lhsT, h_prev[:, BC * k:BC * k + BC],
                       start=(k == 0), stop=(k == c))
                act(hl[:, BC * c:BC * c + BC], z, E.Tanh,
                    bias=sm(f"b{l}")[:, c:c + 1], scale=s[:, c:c + 1])
                mm(zg[:, BC * c:BC * c + BC], wdG[:, P * c:P * c + P],
                   D_prev[:, BC * c:BC * c + BC], start=True, stop=True)
                if mid is not None and c == 1:
                    mid()
            hql = scr.tile([P, 4 * BC], fp16, name=f"hq{l}", tag=f"hq{l}")
            act(hql, hl, E.Square)
            Dl = wgt.tile([P, 4 * BC], bf16, name=f"D{l}", tag=f"D{l}")
            # D_l = (h^2 - 1) * zg  (zg = wdG @ D_{l-1}, PSUM fp32)
            stt(Dl, hql, 1.0, zg, op0=ALU.subtract, op1=ALU.mult)
            return hl, Dl

        # s3+s4 batched scale chain, emitted mid-L2 via callback
        s34_box = {}

        def mid_l2():
            ncol3 = ncol_direct(3)
            # layer-4 prep
            v4a = scr.tile([P, 32], f32, name="v4a", tag="v4a")
            mul(v4a, e4t, sm("md4t"))
            v4b = scr.tile([P, 32], f32, name="v4b", tag="v4b")
            mul(v4b, sm("w4t"), sm("mo4t"))
            vt4 = wgt.tile([P, 32], fp16, name="vt4", tag="vt4")
            tt(vt4, v4a, v4b, op=ALU.add)
            vsq4 = scr.tile([P, 32], fp16, name="vsq4", tag="vsq4")
            act(vsq4, vt4, E.Square)
            pn4 = psN.tile([8, 2], f32, name="pn4", tag="pn")
            for k in range(4):
                mm(pn4, vsq4[:, 8 * k:8 * k + 8], ones2,
                   start=(k == 0), stop=(k == 3))
            n34 = scr.tile([P, 5], f32, name="n34", tag="n34")
            nc.vector.memset(n34[:, 4:5], 1.0)   # rows 8.. of the s4 col
            cp(n34[:, 0:4], ncol3)
            cp(n34[0:8, 4:5], pn4[:, 0:1])
            s34 = make_scale(n34, eg34, (P, 5), "l34")
            s34_box["s3"] = s34[:, 0:4]
            s34_box["s4"] = s34[0:8, 4:5]
            s34_box["v4a"] = v4a
            s34_box["vt4"] = vt4

        h2, D2 = big_batch(2, s2, wdG2, h1, D1, mid=mid_l2)
        s3, s4 = s34_box["s3"], s34_box["s4"]
        v4a, vt4 = s34_box["v4a"], s34_box["vt4"]
        wdG3 = make_wdG(3, s2)

        h3, D3 = big_batch(3, s3, wdG3, h2, D2)

        vd4 = wgt.tile([P, 32], bf16, name="vd4", tag="vd4")
        for k in range(4):
            ts(vd4[:, 8 * k:8 * k + 8], v4a[:, 8 * k:8 * k + 8],
               s3[:, k:k + 1], None, op0=ALU.mult)

        # ================= layer 4 batch ==================================
        pz4 = pzc.tile([8, BC], f32, name="pz4", tag="ps")
        for k in range(4):
            mm(pz4, vt4[:, 8 * k:8 * k + 8], h3[:, BC * k:BC * k + BC],
               start=(k == 0), stop=(k == 3))
        h4 = wgt.tile([8, BC], f32, name="h4", tag="h4")
        act(h4, pz4, E.Tanh, bias=b4, scale=s4)
        nc.sync.dma_start(t["hT_out"], h4)
        pf4 = pzc.tile([8, BC], f32, name="pf4", tag="ps")
        for k in range(4):
            mm(pf4, vd4[:, 8 * k:8 * k + 8], D3[:, BC * k:BC * k + BC],
               start=(k == 0), stop=(k == 3))
        hq4 = scr.tile([8, BC], f32, name="hq4", tag="hq4")
        mul(hq4, h4, h4)
        s24 = scr.tile([8, BC], f32, name="s24", tag="s24")
        ts(s24, hq4, 1.0, 1.0, op0=ALU.mult, op1=ALU.subtract)   # h4^2 - 1
        gt = scr.tile([8, BC], f32, name="gt", tag="gt")
        stt(gt, pf4, s4, s24, op0=ALU.mult, op1=ALU.mult)
        # fast natural log: ln(x) ~= A*bits(x) + B   (max abs err ~0.03)
        gbits = scr.tile([8, BC], f32, name="gbits", tag="gbits")
        cp(gbits, gt.bitcast(i32))
        sld = wgt.tile([8, BC], f32, name="sld", tag="sld")
        ts(sld, gbits, FASTLN_A, FASTLN_B, op0=ALU.mult, op1=ALU.add)
        nc.gpsimd.dma_start(t["sldT_out"], sld)

    nc.compile()
    return nc


def _host_prep(x, W1, logg1, bias1, W2, logg2, bias2, W3, logg3, bias3,
               W4, logg4, bias4):
    """Pure layout prep (transpose / reshape / gather / masks / casts)."""
    f = np.float32

    def cols(a):          # [512]-ish vector -> [128, 4] column-chunk layout
        return np.ascontiguousarray(np.reshape(a, (4, P)).T).astype(f)

    smalls = np.zeros((P, SMALL_W), f)

    def put(name, arr):
        a, b = _SM[name]
        smalls[:arr.shape[0], a:b] = arr

    def fold(m):          # [512, 8] -> [128, (k x)] with k = row-chunk
        return m.reshape(4, P, 8).transpose(1, 0, 2).reshape(P, 32)

    put("ident", np.eye(P, dtype=f))
    put("w1n", fold(W1))                                   # natural [512,8]
    put("w4t", fold(np.ascontiguousarray(W4.T)))           # [512,8]
    put("w1dg", cols(W1[np.arange(H), np.arange(H) // 64]))
    put("w4dg", cols(W4[np.arange(H) // 64, np.arange(H)]))
    put("lg1", cols(logg1)); put("b1", cols(bias1))
    put("lg2", cols(logg2)); put("b2", cols(bias2))
    put("lg3", cols(logg3)); put("b3", cols(bias3))
    put("lg4c", logg4.reshape(8, 1).astype(f))
    put("b4c", bias4.reshape(8, 1).astype(f))
    # structural masks
    o = np.arange(H)[:, None] // 64
    i1 = np.arange(8)[None, :]
    md1 = (i1 == o).astype(f); mo1 = (i1 < o).astype(f)    # [512, 8] natural
    put("md1n", fold(md1)); put("mo1n", fold(mo1))
    ii = np.arange(H)[:, None] // 64
    o4 = np.arange(8)[None, :]
    md4 = (o4 == ii).astype(f); mo4 = (o4 > ii).astype(f)  # [512, 8] transposed
    put("md4t", fold(md4)); put("mo4t", fold(mo4))

    w2T = np.ascontiguousarray(W2.T).astype(np.float16)
    w3T = np.ascontiguousarray(W3.T).astype(np.float16)
    xT = np.ascontiguousarray(x.T).astype(np.float16)      # [8, 2048]
    return xT, w2T, w3T, smalls


def kernel(**inputs):
    global LAST_RESULTS
    from concourse.bass_utils import run_bass_kernel_spmd

    xT, w2T, w3T, smalls = _host_prep(**{k: np.asarray(v) for k, v in inputs.items()})

    if "nc" not in _CACHE:
        _CACHE["nc"] = _build()
    nc = _CACHE["nc"]

    in_maps = []
    for c in range(NCORE):
        in_maps.append({
            "xT": np.ascontiguousarray(xT[:, BC * c:BC * (c + 1)]),
            "w2T": w2T, "w3T": w3T, "smalls": smalls,
        })
    res = run_bass_kernel_spmd(nc, in_maps, core_ids=list(range(NCORE)),
                               trace=TRACE)
    LAST_RESULTS = res

    B = BC * NCORE
    h = np.empty((B, 8), np.float32)
    sld = np.empty((B, 8), np.float32)
    for c, r in enumerate(res.results):
        h[BC * c:BC * (c + 1)] = r["hT_out"].T
        sld[BC * c:BC * (c + 1)] = r["sldT_out"].T
    return h, sld
